# revision 1
# baseline (speedup 1.0000x reference)
"""CGCNNConv fused kernel for 8x Trainium2 NeuronCores.

Strategy (edge-parallel, owner-sorted):
- Edges are assigned to the core that owns their src node (node range shard),
  sorted by src, grouped into 49 windows of 128 nodes, each window padded to
  T_W=18 tiles of 128 edges (dummy edges hit all-zero table rows and a
  non-matching one-hot row, so they contribute exactly zero).
- Per core, on device:
  Phase 1: build fp16 projection tables via PE matmuls from host-provided h^T:
     S_loc[n, 0:256]  = [-(h Wg_src) - gb | h Wc_src + cb]   (local 6250 rows)
     T_lo / T_hi[n, :] = [-(h Wg_dst) | h Wc_dst]            (25000 rows each)
  Phase 2: per window: dma_gather rows S_loc[src], T_lo[dst], T_hi[dst]
     (lo/hi split keeps indices within int16; misses hit a zero row),
     preact = gather_S + gather_Tlo + gather_Thi + ef^T @ W_ef  (PE identity-
     add matmuls accumulate everything in PSUM),
     gate (negated) and cand halves through exp/ln-only activations:
       E = exp(preact); U = ln(1+E); G = exp(-U_gate); m = G * U_cand
     scatter-add via one-hot matmul into the window's PSUM accumulator.
  Phase 3: BN stats (partial sums -> 1KB AllReduce), scale/shift, residual,
     softplus, write the core's output slice.
- Host assembles the 8 output slices.
"""

import numpy as np

N_NODES = 50000
N_EDGES = 800000
D = 128
DE = 10
NCORES = 8
NB = N_NODES // NCORES          # 6250 nodes per core
NW = 49                         # windows of 128 nodes (49*128 = 6272 >= 6250)
TW = 18                         # tiles of 128 edges per window
WEDGE = TW * 128                # 2176 edges per window
E_PAD = NW * WEDGE              # 106624 padded edges per core
NLO = 25000                     # T table split point
TROWS = 196 * 128               # 25088 rows per T table (>= NLO + zero row)
SROWS = NW * 128                # 6272 rows in local S table
BN_EPS = 1e-5


def _wrap_idx(flat16):
    """dma_gather index layout: flat[k] -> partition k%16 (replicated x8), free k//16."""
    n = flat16.shape[0]
    arr = flat16.reshape(n // 16, 16).T          # [16, n/16]
    return np.tile(arr, (8, 1))                  # [128, n/16] int16


def _prep_core(k, src, dst, ef_t):
    """Build one core's padded edge ordering + gather/scatter index arrays."""
    base = k * NB
    sel = np.where((src >= base) & (src < base + NB))[0]
    order = np.argsort(src[sel], kind="stable")
    sel = sel[order]
    s_loc = src[sel] - base                      # [Ek] in [0, NB)
    d_glob = dst[sel]

    sgi = np.full(E_PAD, NB, dtype=np.int16)     # S zero row = NB (6250)
    tlo = np.full(E_PAD, NLO, dtype=np.int16)    # T zero rows = 25000
    thi = np.full(E_PAD, NLO, dtype=np.int16)
    wloc = np.full(E_PAD, -512.0, dtype=np.float16)
    eft_pad = np.zeros((DE, E_PAD), dtype=np.float16)

    win = s_loc // 128
    bounds = np.searchsorted(win, np.arange(NW + 1))
    for w in range(NW):
        lo, hi = bounds[w], bounds[w + 1]
        cnt = hi - lo
        assert cnt <= WEDGE, f"window overflow: core {k} win {w} cnt {cnt}"
        p0 = w * WEDGE
        sgi[p0:p0 + cnt] = s_loc[lo:hi].astype(np.int16)
        dw = d_glob[lo:hi]
        is_lo = dw < NLO
        tlo[p0:p0 + cnt] = np.where(is_lo, dw, NLO).astype(np.int16)
        thi[p0:p0 + cnt] = np.where(is_lo, NLO, dw - NLO).astype(np.int16)
        wloc[p0:p0 + cnt] = (s_loc[lo:hi] - 128 * w).astype(np.float16)
        eft_pad[:, p0:p0 + cnt] = ef_t[:, sel[lo:hi]]

    # pack per window: [sgi | tlo | thi (int16 wrapped) | wloc (f16)] as int32
    wloc_w = wloc.reshape(NW * TW, 128).T.astype(np.float16)     # [128, NW*TW]
    blocks = []
    for w in range(NW):
        sl = slice(w * WEDGE, (w + 1) * WEDGE)
        blk = np.concatenate([
            _wrap_idx(sgi[sl]).view(np.int32),
            _wrap_idx(tlo[sl]).view(np.int32),
            _wrap_idx(thi[sl]).view(np.int32),
            np.ascontiguousarray(wloc_w[:, w * TW:(w + 1) * TW]).view(np.int32),
        ], axis=1)
        blocks.append(blk)
    idx_all = np.concatenate(blocks, axis=1)   # [128, NW*(3*WEDGE//32 + TW//2)]
    return idx_all.copy(), eft_pad


def _setup_act_tables():
    """Point walrus at an act_info.json containing only the combined
    natural_log_exp set, so every Exp/Ln lands in ONE table set (one
    ACT_TABLE_LOAD total). Default set selection picks exp_and_others +
    natural_log, reloading tables at every Exp<->Ln transition (~2.7us each,
    ~1800 times in this kernel)."""
    import os, json, glob, shutil, tempfile
    if os.environ.get("BASS_ACT_ROOT_JSON_PATH"):
        return
    import neuronxcc
    cand = glob.glob(os.path.join(os.path.dirname(neuronxcc.__file__),
                                  "pwp", "pwp_bin_*", "act_info.json"))
    srcj = None
    for c in cand:
        d = json.load(open(c))
        names = [s.get("name") for s in d.get("act_func_sets", [])]
        if "natural_log_exp_and_others" in names:
            srcj = c
            break
    if srcj is None:
        return
    dstdir = os.path.join(tempfile.gettempdir(), "act_nlexp_only")
    os.makedirs(dstdir, exist_ok=True)
    d = json.load(open(srcj))
    keep = [s for s in d["act_func_sets"] if s["name"] == "natural_log_exp_and_others"]
    d["act_func_sets"] = keep
    srcdir = os.path.dirname(srcj)
    for s in keep:
        for key in ("bkt_bin", "ctrl_bin", "profile_json"):
            f = s.get(key)
            if f and not os.path.exists(os.path.join(dstdir, f)):
                shutil.copy(os.path.join(srcdir, f), os.path.join(dstdir, f))
    for extra in d.get("pwp_file_keys", []):
        pass
    for f in glob.glob(os.path.join(srcdir, "*.bin")) + glob.glob(os.path.join(srcdir, "*.json")):
        b = os.path.basename(f)
        if b != "act_info.json" and not os.path.exists(os.path.join(dstdir, b)):
            try:
                os.symlink(f, os.path.join(dstdir, b))
            except OSError:
                pass
    with open(os.path.join(dstdir, "act_info.json"), "w") as fh:
        json.dump(d, fh)
    os.environ["BASS_ACT_ROOT_JSON_PATH"] = os.path.join(dstdir, "act_info.json")

    # bass-side set selection (insert_act_table_loads) must see the same
    # single-set table list so the pre-placed act_func_set_id matches.
    import concourse.hw_specs as hw_specs
    import concourse.bacc as bacc_mod
    import concourse.mybir as mybir
    tables = {keep[0]["name"]: {mybir.ActivationFunctionType.from_pwp(v)
                                for v in keep[0]["act"].keys()}}

    def _patched(module_arch):
        return tables
    hw_specs.get_activation_tables = _patched
    bacc_mod.get_activation_tables = _patched


def _build_nc():
    import concourse.bass as bass
    import concourse.bacc as bacc
    import concourse.mybir as mybir
    import concourse.tile as tile
    from concourse.masks import make_identity

    f16, f32, i32, i16 = (mybir.dt.float16, mybir.dt.float32,
                          mybir.dt.int32, mybir.dt.int16)
    AF = mybir.ActivationFunctionType
    OP = mybir.AluOpType
    P = 128

    nc = bacc.Bacc("TRN2", target_bir_lowering=False, debug=False,
                   num_devices=NCORES, num_swdge_queues=3)

    hT = nc.dram_tensor("hT", [P, 50176], f16, kind="ExternalInput")
    hTs = nc.dram_tensor("hTs", [P, SROWS], f16, kind="ExternalInput")
    wsrc = nc.dram_tensor("wsrc", [P, 256], f16, kind="ExternalInput")
    wdst = nc.dram_tensor("wdst", [P, 256], f16, kind="ExternalInput")
    wef = nc.dram_tensor("wef", [DE, 256], f16, kind="ExternalInput")
    bias = nc.dram_tensor("bias", [P, 256], f32, kind="ExternalInput")
    eft = nc.dram_tensor("eft", [DE, E_PAD], f16, kind="ExternalInput")
    IWB = 3 * (WEDGE // 32) + TW // 2   # int32 cols per window block
    idx_all = nc.dram_tensor("idx_all", [P, NW * IWB], i32, kind="ExternalInput")
    hres = nc.dram_tensor("hres", [SROWS, D], f32, kind="ExternalInput")
    bng = nc.dram_tensor("bng", [1, D], f32, kind="ExternalInput")
    bnb = nc.dram_tensor("bnb", [1, D], f32, kind="ExternalInput")
    out_d = nc.dram_tensor("out", [SROWS, D], f32, kind="ExternalOutput")

    IW = WEDGE // 32   # int32 cols per window of idx input (68)

    with tile.TileContext(nc) as tc:
        with (
            tc.tile_pool(name="const", bufs=1) as cp,
            tc.tile_pool(name="tabl", bufs=3) as tp,
            tc.tile_pool(name="edge", bufs=2) as ep,
            tc.tile_pool(name="act", bufs=3) as ap_,
            tc.tile_pool(name="psA", bufs=4, space="PSUM") as ppa,
            tc.tile_pool(name="psB", bufs=2, space="PSUM") as ppb,
            tc.tile_pool(name="dram", bufs=1, space="DRAM") as dp,
        ):
            # ---------- constants ----------
            ident = cp.tile([P, P], f16)
            make_identity(nc, ident[:])
            iota_i = cp.tile([P, P], i16)
            nc.gpsimd.iota(iota_i[:], pattern=[[1, P]], base=0, channel_multiplier=0)
            iota_f = cp.tile([P, P], f16)
            nc.vector.tensor_copy(iota_f[:], iota_i[:])
            ones_c = cp.tile([P, 1], f32)
            nc.vector.memset(ones_c[:], 1.0)

            wsrc_s = cp.tile([P, 256], f16)
            nc.sync.dma_start(wsrc_s[:], wsrc[:])
            wdst_s = cp.tile([P, 256], f16)
            nc.sync.dma_start(wdst_s[:], wdst[:])
            wef_s = cp.tile([DE, 256], f16)
            nc.sync.dma_start(wef_s[:], wef[:])
            bias_s = cp.tile([P, 256], f32)
            nc.sync.dma_start(bias_s[:], bias[:])
            zrow = cp.tile([1, 256], f16)
            nc.vector.memset(zrow[:], 0.0)

            # ---------- phase 1: tables ----------
            s_tab = dp.tile([SROWS, 256], f16)
            tlo_tab = dp.tile([TROWS, 256], f16)
            thi_tab = dp.tile([TROWS, 256], f16)

            for i4 in range((NW + 3) // 4):  # S local table, bias folded
                n4 = min(4, NW - i4 * 4)
                hbig = tp.tile([P, 4 * P], f16, tag="hbig")
                nc.sync.dma_start(hbig[:, :n4 * P],
                                  hTs[:, i4 * 4 * P: i4 * 4 * P + n4 * P])
                for j in range(n4):
                    i = i4 * 4 + j
                    ps = ppb.tile([P, 256], f32, tag="genps")
                    nc.tensor.matmul(ps[:], lhsT=hbig[:, j * P:(j + 1) * P], rhs=wsrc_s[:], start=True, stop=True)
                    row = tp.tile([P, 256], f16, tag="srow")
                    nc.vector.tensor_tensor(row[:], ps[:], bias_s[:], op=OP.add)
                    nc.sync.dma_start(s_tab[i * P:(i + 1) * P, :], row[:])
            nc.sync.dma_start(s_tab[NB:NB + 1, :], zrow[:])

            for half, tab in ((0, tlo_tab), (1, thi_tab)):
                for i4 in range(TROWS // (4 * P)):
                    hbig = tp.tile([P, 4 * P], f16, tag="hbig")
                    nc.sync.dma_start(hbig[:], hT[:, half * NLO + i4 * 4 * P: half * NLO + (i4 + 1) * 4 * P])
                    for j in range(4):
                        i = i4 * 4 + j
                        ps = ppb.tile([P, 256], f32, tag="genps")
                        nc.tensor.matmul(ps[:], lhsT=hbig[:, j * P:(j + 1) * P], rhs=wdst_s[:], start=True, stop=True)
                        row = tp.tile([P, 256], f16, tag="srow")
                        if i % 2 == 0:
                            nc.vector.tensor_copy(row[:], ps[:])
                        else:
                            nc.scalar.copy(row[:], ps[:])
                        nc.sync.dma_start(tab[i * P:(i + 1) * P, :], row[:])
            nc.sync.dma_start(tlo_tab[NLO:NLO + 1, :], zrow[:])

            # ---------- phase 2: edges ----------
            agg = cp.tile([P, NW, D], f32)         # [node%128, window, j]
            rstat = cp.tile([P, 256], f32)         # [sum | sumsq] accumulators
            nc.vector.memset(rstat[:], 0.0)

            for w in range(NW):
                ia = ep.tile([P, IWB], i32, tag="ia")
                nc.sync.dma_start(ia[:], idx_all[:, w * IWB:(w + 1) * IWB])
                si = ia[:, 0:IW]
                li = ia[:, IW:2 * IW]
                hi = ia[:, 2 * IW:3 * IW]
                wl = ia[:, 3 * IW:IWB].bitcast(f16)
                efts = ep.tile([DE, WEDGE], f16, tag="efts")
                nc.sync.dma_start(efts[:], eft[:, w * WEDGE:(w + 1) * WEDGE])

                zs = ep.tile([P, TW, 256], f16, tag="zs")
                nc.gpsimd.dma_gather(zs[:], s_tab[:], si.bitcast(i16),
                                     WEDGE, WEDGE, 256, single_packet=False,
                                     queue_num=0)
                zlo = ep.tile([P, TW, 256], f16, tag="zlo")
                nc.gpsimd.dma_gather(zlo[:], tlo_tab[:], li.bitcast(i16),
                                     WEDGE, WEDGE, 256, single_packet=False,
                                     queue_num=1)
                zhi = ep.tile([P, TW, 256], f16, tag="zhi")
                nc.gpsimd.dma_gather(zhi[:], thi_tab[:], hi.bitcast(i16),
                                     WEDGE, WEDGE, 256, single_packet=False,
                                     queue_num=2)
                zt = ep.tile([P, TW, 256], f16, tag="zt")
                nc.vector.tensor_tensor(zt[:], zlo[:], zhi[:], op=OP.add)

                pw = ppb.tile([P, D], f32, tag="winps")

                for c0 in range(0, TW, 2):
                    pp = ppa.tile([P, 2, 256], f32, tag="pp")
                    for j in range(2):
                        s = c0 + j
                        nc.tensor.matmul(pp[:, j, :], lhsT=efts[:, s * P:(s + 1) * P],
                                         rhs=wef_s[:], start=True, stop=False)
                        nc.tensor.matmul(pp[:, j, :], lhsT=ident[:], rhs=zs[:, s, :],
                                         start=False, stop=False)
                        nc.tensor.matmul(pp[:, j, :], lhsT=ident[:], rhs=zt[:, s, :],
                                         start=False, stop=True)
                    e16 = ap_.tile([P, 512], f16, tag="e16")
                    nc.scalar.activation(e16[:], pp[:], AF.Exp)
                    u16 = ap_.tile([P, 512], f16, tag="u16")
                    nc.scalar.activation(u16[:], e16[:], AF.Ln, bias=1.0)
                    for j in range(2):
                        s = c0 + j
                        g16 = ap_.tile([P, D], f16, tag="g16")
                        nc.scalar.activation(g16[:], u16[:, j * 256:j * 256 + D],
                                             AF.Exp, scale=-1.0)
                        m16 = ap_.tile([P, D], f16, tag="m16")
                        nc.vector.tensor_tensor(m16[:], g16[:],
                                                u16[:, j * 256 + D:(j + 1) * 256], op=OP.mult)
                        oh = ap_.tile([P, P], f16, tag="oh")
                        nc.vector.tensor_tensor(oh[:], iota_f[:],
                                                wl[:, s:s + 1].to_broadcast([P, P]),
                                                op=OP.is_equal)
                        nc.tensor.matmul(pw[:], lhsT=oh[:], rhs=m16[:],
                                         start=(s == 0), stop=(s == TW - 1))

                nc.vector.tensor_copy(agg[:, w, :], pw[:])
                sq = ap_.tile([P, D], f32, tag="sq")
                nc.vector.tensor_tensor(sq[:], agg[:, w, :], agg[:, w, :], op=OP.mult)
                nc.vector.tensor_tensor(rstat[:, 0:D], rstat[:, 0:D], agg[:, w, :], op=OP.add)
                nc.vector.tensor_tensor(rstat[:, D:256], rstat[:, D:256], sq[:], op=OP.add)

            # ---------- phase 3: BN stats + output ----------
            pstat = ppb.tile([1, 256], f32, tag="genps")
            nc.tensor.matmul(pstat[:], lhsT=ones_c[:], rhs=rstat[:], start=True, stop=True)
            stat_l = cp.tile([1, 256], f32)
            nc.vector.tensor_copy(stat_l[:], pstat[:])

            cc_in = dp.tile([1, 256], f32)
            cc_out = dp.tile([1, 256], f32)
            nc.gpsimd.dma_start(cc_in[:], stat_l[:])
            nc.gpsimd.collective_compute(
                "AllReduce", OP.add,
                replica_groups=[list(range(NCORES))],
                ins=[cc_in.opt()], outs=[cc_out.opt()])
            stat_g = cp.tile([1, 256], f32)
            nc.sync.dma_start(stat_g[:], cc_out[:])

            bng_s = cp.tile([1, D], f32)
            nc.sync.dma_start(bng_s[:], bng[:])
            bnb_s = cp.tile([1, D], f32)
            nc.sync.dma_start(bnb_s[:], bnb[:])

            mean = cp.tile([1, D], f32)
            nc.vector.tensor_scalar_mul(mean[:], stat_g[:, 0:D], 1.0 / N_NODES)
            ex2 = cp.tile([1, D], f32)
            nc.vector.tensor_scalar_mul(ex2[:], stat_g[:, D:256], 1.0 / N_NODES)
            msq = cp.tile([1, D], f32)
            nc.vector.tensor_tensor(msq[:], mean[:], mean[:], op=OP.mult)
            var = cp.tile([1, D], f32)
            nc.vector.tensor_tensor(var[:], ex2[:], msq[:], op=OP.subtract)
            vpe = cp.tile([1, D], f32)
            nc.vector.tensor_scalar_add(vpe[:], var[:], BN_EPS)
            lnv = cp.tile([1, D], f32)
            nc.scalar.activation(lnv[:], vpe[:], AF.Ln)
            rstd = cp.tile([1, D], f32)
            nc.scalar.activation(rstd[:], lnv[:], AF.Exp, scale=-0.5)
            scale_r = cp.tile([1, D], f32)
            nc.vector.tensor_tensor(scale_r[:], bng_s[:], rstd[:], op=OP.mult)
            mscl = cp.tile([1, D], f32)
            nc.vector.tensor_tensor(mscl[:], mean[:], scale_r[:], op=OP.mult)
            shift_r = cp.tile([1, D], f32)
            nc.vector.tensor_tensor(shift_r[:], bnb_s[:], mscl[:], op=OP.subtract)

            sc_t = cp.tile([P, D], f32)
            nc.gpsimd.partition_broadcast(sc_t[:], scale_r[:])
            sh_t = cp.tile([P, D], f32)
            nc.gpsimd.partition_broadcast(sh_t[:], shift_r[:])

            for w in range(NW):
                ht = tp.tile([P, D], f32, tag="hrt")
                nc.sync.dma_start(ht[:], hres[w * P:(w + 1) * P, :])
                t1 = tp.tile([P, D], f32, tag="t1")
                nc.vector.tensor_tensor(t1[:], agg[:, w, :], sc_t[:], op=OP.mult)
                nc.vector.tensor_tensor(t1[:], t1[:], sh_t[:], op=OP.add)
                nc.vector.tensor_tensor(t1[:], t1[:], ht[:], op=OP.add)
                t2 = tp.tile([P, D], f32, tag="t2")
                nc.scalar.activation(t2[:], t1[:], AF.Exp)
                t3 = tp.tile([P, D], f32, tag="t3")
                nc.scalar.activation(t3[:], t2[:], AF.Ln, bias=1.0)
                nc.sync.dma_start(out_d[w * P:(w + 1) * P, :], t3[:])

    nc.compile()
    return nc


_NC_CACHE = None


def kernel(h, edge_index, edge_feat, gate_w, gate_b, cand_w, cand_b,
           bn_gamma, bn_beta):
    global _NC_CACHE
    from concourse.bass_utils import run_bass_kernel_spmd

    h = np.asarray(h, dtype=np.float32)
    ei = np.asarray(edge_index)
    src = ei[0].astype(np.int64)
    dst = ei[1].astype(np.int64)
    ef = np.asarray(edge_feat, dtype=np.float32)
    gw = np.asarray(gate_w, dtype=np.float32)
    gb = np.asarray(gate_b, dtype=np.float32)
    cw = np.asarray(cand_w, dtype=np.float32)
    cb = np.asarray(cand_b, dtype=np.float32)
    gam = np.asarray(bn_gamma, dtype=np.float32).reshape(1, D)
    bet = np.asarray(bn_beta, dtype=np.float32).reshape(1, D)

    # weight layouts (gate half negated so exp(-a) comes straight from PSUM)
    wsrc = np.concatenate([-gw[0:D], cw[0:D]], axis=1).astype(np.float16)         # [128, 256]
    wdst = np.concatenate([-gw[D:2 * D], cw[D:2 * D]], axis=1).astype(np.float16)
    wef_h = np.concatenate([-gw[2 * D:], cw[2 * D:]], axis=1).astype(np.float16)  # [10, 256]
    bias = np.concatenate([-gb, cb]).astype(np.float32)[None, :].repeat(128, 0)   # [128, 256]

    hT16 = np.zeros((D, 50176), dtype=np.float16)
    hT16[:, :N_NODES] = h.T.astype(np.float16)
    ef_t = ef.T.astype(np.float16)                                                # [10, E]

    in_maps = []
    for k in range(NCORES):
        idx_all_k, eft_pad = _prep_core(k, src, dst, ef_t)
        base = k * NB
        hTs16 = np.zeros((D, SROWS), dtype=np.float16)
        hTs16[:, :NB] = h.T[:, base:base + NB].astype(np.float16)
        hres = np.zeros((SROWS, D), dtype=np.float32)
        hres[:NB] = h[base:base + NB]
        in_maps.append({
            "hT": hT16, "hTs": hTs16, "wsrc": wsrc, "wdst": wdst,
            "wef": wef_h, "bias": bias, "eft": eft_pad,
            "idx_all": idx_all_k,
            "hres": hres, "bng": gam, "bnb": bet,
        })

    _setup_act_tables()
    if _NC_CACHE is None:
        _NC_CACHE = _build_nc()
    res = run_bass_kernel_spmd(_NC_CACHE, in_maps, core_ids=list(range(NCORES)))
    out = np.concatenate([res.results[k]["out"][:NB] for k in range(NCORES)], axis=0)
    return out.astype(np.float32)


if __name__ == "__main__":
    import jax
    import reference
    cpu = jax.devices("cpu")[0]
    with jax.default_device(cpu):
        ins = reference.setup_inputs()
        ins = {k: np.asarray(v) for k, v in ins.items()}
        exp = np.asarray(reference.reference(**{k: jax.device_put(v, cpu) for k, v in ins.items()}))
    got = kernel(**ins)
    err = np.abs(got - exp).max() / np.abs(exp).max()
    print("rel err:", err)



# revision 15
# speedup vs baseline: 5.3710x; 5.3710x over previous
"""CGCNNConv fused kernel for 8x Trainium2 NeuronCores (v2.1).

Measured bottleneck on this stack is per-execute input binding (~1-2 ms per
bound tensor + ~0.6 ms/MB), not compute, so v2 ships ONE packed f16 input
tensor (~6.5 MB) + one f16 output per core and reconstructs everything else
on device:

- Edges are owner-sorted by src into 49 windows of 128 nodes; within a
  window, edges whose hfull row < 25088 ("lo", node id < 25000) come first,
  then "hi" edges, each region padded to whole 128-edge tiles (TL[w]/TH[w]
  tiles, data-derived compile-time constants, max over cores).
- Phase 0: copy the local h slice ([6272,128] f16, 22 zero pad rows) to
  hloc, AllGather hloc into hfull [50176,128].
- Phase 2 per window: three dma_gathers (transpose=True) fetch raw h rows
  transposed: src rows from hloc (local idx), dst rows from hfull lo/hi
  halves (keeps idx in int16; pads hit zero rows). The [128d, edge] layout
  feeds projection matmuls directly:
    preact[e,0:256] = ef_e @ Wef + bias (ones row) + h_dst @ Wdst' + h_src @ Wsrc'
  with W' = [-Wgate | Wcand] so the gate is pre-negated. Per 4-tile group:
  E = exp(preact) [ACT]; cand u = ln1p(E_c) [ACT]; gate sigma =
  recip_approx(1+E_g) [DVE]; m = sigma*u; scatter-add via one-hot matmul
  (one-hots built 4 tiles per is_equal) into the window PSUM accumulator.
- Phase 3: BN stats partial sums -> 1KB AllReduce -> scale/shift + residual
  (f16 h) + softplus(exp/ln1p), f16 output.

Activations use only Exp/Ln so a single act table set is loaded once
(_setup_act_tables pins natural_log_exp_and_others).
"""

import numpy as np

N_NODES = 50000
N_EDGES = 800000
D = 128
DE = 10
NCORES = 8
NB = N_NODES // NCORES          # 6250 nodes per core
NW = 49                         # windows of 128 src nodes
HROWS = NW * 128                # 6272 rows in the padded h slice
NLO = 25088                     # lo/hi split in hfull ROW space (4*6272)
NSPLIT = 25000                  # same split in node-id space (4*6250)
PAD_LO = 6250                   # zero row in [0,25088) (core 0 pad row)
PAD_HI = 6250                   # zero row idx within hi range (abs 31338)
PAD_SRC = 6250                  # zero row in hloc
BN_EPS = 1e-5
_GATHER_QUEUES = (0, 0, 0)      # 3-queue round-robin trips a tile DMASW sem/queue mismatch

_NC_CACHE = None
_LAYOUT_CACHE = None
_LAST_IN_MAPS = None
_DEBUG = False                  # adds agg/stat debug outputs to the program


def _hrow(n):
    """hfull row of global node n (per-core slices carry 22 pad rows)."""
    return HROWS * (n // NB) + n % NB


def _wrap_idx(flat16):
    """dma_gather index layout: flat[k] -> partition k%16 (replicated x8)."""
    n = flat16.shape[0]
    arr = flat16.reshape(n // 16, 16).T          # [16, n/16]
    return np.tile(arr, (8, 1))                  # [128, n/16] int16


def _groups(tw):
    out = []
    s = 0
    while s < tw:
        out.append((s, min(4, tw - s)))
        s += 4
    return out


def _layout(TLs, THs):
    """Row layout of the packed stream tensor. All offsets in 512-elem rows."""
    lay = {}
    r = 0
    lay["wmix"] = r; r += 128            # [128, 512]
    lay["wefx"] = r; r += 11             # [11, 256] (cols 256:512 unused)
    lay["bn"] = r; r += 1                # [1, 512] f16 = [1,256] f32 bitcast
    lay["h"] = r; r += HROWS * D // 512  # 1568 rows
    win = []
    for w in range(NW):
        tl, th = TLs[w], THs[w]
        tw = tl + th
        ne = tw * 128
        nep = ((ne + 511) // 512) * 512
        twp = ((tw + 3) // 4) * 4
        ic = tw * 16 + twp                       # [128, ic]: il|ih|isrc|wl
        ent = {"tl": tl, "th": th, "tw": tw, "ne": ne, "nep": nep,
               "twp": twp, "ic": ic}
        ent["ix"] = r; r += ic * 128 // 512
        ent["eftx"] = r; r += 11 * nep // 512    # [11, nep] f16
        win.append(ent)
    lay["win"] = win
    lay["rows"] = r
    return lay


def _build_nc(lay):
    import concourse.bass as bass
    import concourse.bacc as bacc
    import concourse.mybir as mybir
    import concourse.tile as tile

    f16, f32, i16 = mybir.dt.float16, mybir.dt.float32, mybir.dt.int16
    AF = mybir.ActivationFunctionType
    OP = mybir.AluOpType
    P = 128
    R = lay["rows"]
    NEMAX = max(e["ne"] for e in lay["win"])
    NEPMAX = max(e["nep"] for e in lay["win"])
    ICMAX = max(e["ic"] for e in lay["win"])

    nc = bacc.Bacc("TRN2", target_bir_lowering=False, debug=False,
                   num_devices=NCORES, num_swdge_queues=3)

    stream = nc.dram_tensor("stream", [R, 512], f16, kind="ExternalInput")
    out_d = nc.dram_tensor("out", [HROWS, D], f16, kind="ExternalOutput")
    if _DEBUG:
        dbg_agg = nc.dram_tensor("dbg_agg", [HROWS, D], f32, kind="ExternalOutput")
        dbg_stat = nc.dram_tensor("dbg_stat", [2, 256], f32, kind="ExternalOutput")

    with tile.TileContext(nc) as tc:
        with (
            tc.tile_pool(name="const", bufs=1) as cp,
            tc.tile_pool(name="win", bufs=2) as wp,
            tc.tile_pool(name="act", bufs=3) as ap_,
            tc.tile_pool(name="psA", bufs=2, space="PSUM") as ppa,
            tc.tile_pool(name="psW", bufs=2, space="PSUM") as ppw,
            tc.tile_pool(name="psC", bufs=1, space="PSUM") as ppc,
            tc.tile_pool(name="dram", bufs=1, space="DRAM") as dp,
        ):
            # ---------- constants ----------
            hloc = dp.tile([HROWS, D], f16)
            hfull = dp.tile([NCORES * HROWS, D], f16)
            iota4_i = cp.tile([P, 4, P], i16)
            nc.gpsimd.iota(iota4_i[:], pattern=[[0, 4], [1, P]], base=0,
                           channel_multiplier=0)
            iota4 = cp.tile([P, 4, P], f16)
            nc.vector.tensor_copy(iota4[:], iota4_i[:])
            ones_c = cp.tile([P, 1], f32)
            nc.vector.memset(ones_c[:], 1.0)

            wmix_s = cp.tile([P, 512], f16)
            nc.sync.dma_start(wmix_s[:], stream[lay["wmix"]:lay["wmix"] + 128, :])
            wefx_s = cp.tile([11, 256], f16)
            nc.sync.dma_start(wefx_s[:], stream[lay["wefx"]:lay["wefx"] + 11, 0:256])
            bn_s = cp.tile([1, 512], f16)
            nc.sync.dma_start(bn_s[:], stream[lay["bn"]:lay["bn"] + 1, :])

            # ---------- phase 0: stage h, AllGather ----------
            h0 = lay["h"]
            nc.sync.dma_start(hloc[:, :], stream[h0:h0 + HROWS * D // 512, :])
            nc.gpsimd.collective_compute(
                "AllGather", OP.bypass,
                replica_groups=[list(range(NCORES))],
                ins=[hloc[:, :].opt()],
                outs=[hfull[:, :].opt()])

            # ---------- phase 2: windows ----------
            agg = cp.tile([P, NW, D], f32)
            rstat = cp.tile([P, 256], f32)
            nc.vector.memset(rstat[:], 0.0)

            for w in range(NW):
                ent = lay["win"][w]
                tl, th, tw, ne = ent["tl"], ent["th"], ent["tw"], ent["ne"]

                ix = wp.tile([P, ICMAX], f16, tag="ix")
                nc.sync.dma_start(ix[:, :ent["ic"]],
                                  stream[ent["ix"]:ent["ix"] + ent["ic"] * 128 // 512, :])
                il = ix[:, 0:tl * 8].bitcast(i16)
                ih = ix[:, tl * 8:tw * 8].bitcast(i16)
                isrc = ix[:, tw * 8:tw * 16].bitcast(i16)
                wlt = ix[:, tw * 16:tw * 16 + tw]
                eftx = wp.tile([11, NEPMAX], f16, tag="eftx")
                nc.sync.dma_start(eftx[:, :ent["nep"]],
                                  stream[ent["eftx"]:ent["eftx"] + 11 * ent["nep"] // 512, :])

                zsT = wp.tile([P, 1, NEMAX], f16, tag="zsT")
                nc.gpsimd.dma_gather(zsT[:, :, 0:ne], hloc[:, :], isrc,
                                     ne, ne, D,
                                     transpose=True, single_packet=False,
                                     queue_num=_GATHER_QUEUES[0])
                zhT = wp.tile([P, 1, NEMAX], f16, tag="zhT")
                nc.gpsimd.dma_gather(zhT[:, :, 0:tl * 128], hfull[0:NLO, :], il,
                                     tl * 128, tl * 128, D,
                                     transpose=True, single_packet=False,
                                     queue_num=_GATHER_QUEUES[1])
                nc.gpsimd.dma_gather(zhT[:, :, tl * 128:ne], hfull[NLO:2 * NLO, :], ih,
                                     th * 128, th * 128, D,
                                     transpose=True, single_packet=False,
                                     queue_num=_GATHER_QUEUES[2])

                pw = ppw.tile([P, D], f32, tag="pw")

                for s0, gs in _groups(tw):
                    oh4 = ap_.tile([P, 4, P], f16, tag="oh4")
                    nc.vector.tensor_tensor(
                        oh4[:, :gs, :], iota4[:, :gs, :],
                        wlt[:, s0:s0 + gs].to_broadcast([P, gs, P]),
                        op=OP.is_equal)
                    pp = ppa.tile([P, 4, 256], f32, tag="pp")
                    for j in range(gs):
                        s = s0 + j
                        nc.tensor.matmul(pp[:, j, :],
                                         lhsT=eftx[:, s * P:(s + 1) * P],
                                         rhs=wefx_s[:], start=True, stop=False)
                        nc.tensor.matmul(pp[:, j, :],
                                         lhsT=zhT[:, 0, s * P:(s + 1) * P],
                                         rhs=wmix_s[:, 256:512],
                                         start=False, stop=False)
                        nc.tensor.matmul(pp[:, j, :],
                                         lhsT=zsT[:, 0, s * P:(s + 1) * P],
                                         rhs=wmix_s[:, 0:256],
                                         start=False, stop=True)

                    e16 = ap_.tile([P, 4, 256], f16, tag="e16")
                    nc.scalar.activation(e16[:, :gs, :], pp[:, :gs, :], AF.Exp)
                    u16 = ap_.tile([P, 4, D], f16, tag="u16")
                    nc.scalar.activation(u16[:, :gs, :], e16[:, :gs, D:256],
                                         AF.Ln, bias=1.0)
                    t32 = ap_.tile([P, 4, D], f32, tag="t32")
                    nc.vector.tensor_scalar_add(t32[:, :gs, :], e16[:, :gs, 0:D], 1.0)
                    r32 = ap_.tile([P, 4, D], f32, tag="r32")
                    nc.vector.reciprocal_approx_fast(r32[:, :gs, :], t32[:, :gs, :])
                    m16 = ap_.tile([P, 4, D], f16, tag="m16")
                    nc.vector.tensor_tensor(m16[:, :gs, :], r32[:, :gs, :],
                                            u16[:, :gs, :], op=OP.mult)
                    for j in range(gs):
                        s = s0 + j
                        nc.tensor.matmul(pw[:], lhsT=oh4[:, j, :], rhs=m16[:, j, :],
                                         start=(s == 0), stop=(s == tw - 1))

                nc.vector.tensor_copy(agg[:, w, :], pw[:])
                sq = ap_.tile([P, D], f32, tag="sq")
                nc.gpsimd.tensor_tensor(sq[:], agg[:, w, :], agg[:, w, :], op=OP.mult)
                nc.vector.tensor_tensor(rstat[:, 0:D], rstat[:, 0:D],
                                        agg[:, w, :], op=OP.add)
                nc.gpsimd.tensor_tensor(rstat[:, D:256], rstat[:, D:256],
                                        sq[:], op=OP.add)

            # ---------- phase 3: BN + residual + softplus ----------
            pstat = ppc.tile([1, 256], f32, tag="misc")
            nc.tensor.matmul(pstat[:], lhsT=ones_c[:], rhs=rstat[:], start=True, stop=True)
            stat_l = cp.tile([1, 256], f32)
            nc.vector.tensor_copy(stat_l[:], pstat[:])

            cc_in = dp.tile([1, 256], f32)
            cc_out = dp.tile([1, 256], f32)
            nc.gpsimd.dma_start(cc_in[:], stat_l[:])
            nc.gpsimd.collective_compute(
                "AllReduce", OP.add,
                replica_groups=[list(range(NCORES))],
                ins=[cc_in.opt()], outs=[cc_out.opt()])
            stat_g = cp.tile([1, 256], f32)
            nc.sync.dma_start(stat_g[:], cc_out[:])

            bnf = bn_s[:].bitcast(f32)           # [1, 256] f32: [gamma | beta]
            mean = cp.tile([1, D], f32)
            nc.vector.tensor_scalar_mul(mean[:], stat_g[:, 0:D], 1.0 / N_NODES)
            ex2 = cp.tile([1, D], f32)
            nc.vector.tensor_scalar_mul(ex2[:], stat_g[:, D:256], 1.0 / N_NODES)
            msq = cp.tile([1, D], f32)
            nc.vector.tensor_tensor(msq[:], mean[:], mean[:], op=OP.mult)
            var = cp.tile([1, D], f32)
            nc.vector.tensor_tensor(var[:], ex2[:], msq[:], op=OP.subtract)
            vpe = cp.tile([1, D], f32)
            nc.vector.tensor_scalar_add(vpe[:], var[:], BN_EPS)
            lnv = cp.tile([1, D], f32)
            nc.scalar.activation(lnv[:], vpe[:], AF.Ln)
            rstd = cp.tile([1, D], f32)
            nc.scalar.activation(rstd[:], lnv[:], AF.Exp, scale=-0.5)
            scale_r = cp.tile([1, D], f32)
            nc.vector.tensor_tensor(scale_r[:], bnf[:, 0:D], rstd[:], op=OP.mult)
            mscl = cp.tile([1, D], f32)
            nc.vector.tensor_tensor(mscl[:], mean[:], scale_r[:], op=OP.mult)
            shift_r = cp.tile([1, D], f32)
            nc.vector.tensor_tensor(shift_r[:], bnf[:, D:256], mscl[:], op=OP.subtract)

            sc_t = cp.tile([P, D], f32)
            nc.gpsimd.partition_broadcast(sc_t[:], scale_r[:])
            sh_t = cp.tile([P, D], f32)
            nc.gpsimd.partition_broadcast(sh_t[:], shift_r[:])

            if _DEBUG:
                for w in range(NW):
                    nc.sync.dma_start(dbg_agg[w * P:(w + 1) * P, :], agg[:, w, :])
                nc.sync.dma_start(dbg_stat[0:1, :], stat_l[:])
                nc.sync.dma_start(dbg_stat[1:2, :], stat_g[:])

            h0 = lay["h"]
            for w in range(NW):
                ht = wp.tile([P, D], f16, tag="hrt")
                nc.sync.dma_start(ht[:], stream[h0 + 32 * w:h0 + 32 * (w + 1), :])
                t1 = wp.tile([P, D], f32, tag="t1")
                nc.vector.tensor_tensor(t1[:], agg[:, w, :], sc_t[:], op=OP.mult)
                nc.vector.tensor_tensor(t1[:], t1[:], sh_t[:], op=OP.add)
                nc.vector.tensor_tensor(t1[:], t1[:], ht[:], op=OP.add)
                t2 = wp.tile([P, D], f32, tag="t2")
                nc.scalar.activation(t2[:], t1[:], AF.Exp)
                t3 = wp.tile([P, D], f16, tag="t3")
                nc.scalar.activation(t3[:], t2[:], AF.Ln, bias=1.0)
                nc.sync.dma_start(out_d[w * P:(w + 1) * P, :], t3[:])

    nc.compile()
    return nc


def _setup_act_tables():
    """Pin walrus to an act_info.json with only natural_log_exp_and_others,
    so every Exp/Ln shares ONE table set (single ACT_TABLE_LOAD)."""
    import os, json, glob, shutil, tempfile
    if os.environ.get("BASS_ACT_ROOT_JSON_PATH"):
        return
    import neuronxcc
    cand = glob.glob(os.path.join(os.path.dirname(neuronxcc.__file__),
                                  "pwp", "pwp_bin_*", "act_info.json"))
    srcj = None
    for c in cand:
        d = json.load(open(c))
        names = [s.get("name") for s in d.get("act_func_sets", [])]
        if "natural_log_exp_and_others" in names:
            srcj = c
            break
    if srcj is None:
        return
    dstdir = os.path.join(tempfile.gettempdir(), "act_nlexp_only")
    os.makedirs(dstdir, exist_ok=True)
    d = json.load(open(srcj))
    keep = [s for s in d["act_func_sets"] if s["name"] == "natural_log_exp_and_others"]
    d["act_func_sets"] = keep
    srcdir = os.path.dirname(srcj)
    for s in keep:
        for key in ("bkt_bin", "ctrl_bin", "profile_json"):
            f = s.get(key)
            if f and not os.path.exists(os.path.join(dstdir, f)):
                shutil.copy(os.path.join(srcdir, f), os.path.join(dstdir, f))
    for f in glob.glob(os.path.join(srcdir, "*.bin")) + glob.glob(os.path.join(srcdir, "*.json")):
        b = os.path.basename(f)
        if b != "act_info.json" and not os.path.exists(os.path.join(dstdir, b)):
            try:
                os.symlink(f, os.path.join(dstdir, b))
            except OSError:
                pass
    with open(os.path.join(dstdir, "act_info.json"), "w") as fh:
        json.dump(d, fh)
    os.environ["BASS_ACT_ROOT_JSON_PATH"] = os.path.join(dstdir, "act_info.json")

    import concourse.hw_specs as hw_specs
    import concourse.bacc as bacc_mod
    import concourse.mybir as mybir
    tables = {keep[0]["name"]: {mybir.ActivationFunctionType.from_pwp(v)
                                for v in keep[0]["act"].keys()}}

    def _patched(module_arch):
        return tables
    hw_specs.get_activation_tables = _patched
    bacc_mod.get_activation_tables = _patched


def _prep(src, dst, ef_t):
    """Per-core edge ordering + per-window region tile counts."""
    cores = []
    for k in range(NCORES):
        base = k * NB
        sel = np.where((src >= base) & (src < base + NB))[0]
        s_loc = (src[sel] - base).astype(np.int64)
        d_glob = dst[sel]
        win = s_loc // 128
        is_hi = (d_glob >= NSPLIT).astype(np.int64)
        order = np.lexsort((is_hi, win))
        sel, s_loc, d_glob, win, is_hi = (sel[order], s_loc[order],
                                          d_glob[order], win[order], is_hi[order])
        bounds = np.searchsorted(win, np.arange(NW + 1))
        lo_cnt = np.zeros(NW, np.int64)
        hi_cnt = np.zeros(NW, np.int64)
        for w in range(NW):
            a, b = bounds[w], bounds[w + 1]
            nhi = int(is_hi[a:b].sum())
            hi_cnt[w] = nhi
            lo_cnt[w] = (b - a) - nhi
        cores.append({"sel": sel, "s_loc": s_loc, "d": d_glob,
                      "bounds": bounds, "lo": lo_cnt, "hi": hi_cnt})
    TLs = [0] * NW
    THs = [0] * NW
    for w in range(NW):
        TLs[w] = max(int(max(-(-c["lo"][w] // 128) for c in cores)), 1)
        THs[w] = max(int(max(-(-c["hi"][w] // 128) for c in cores)), 1)
    return TLs, THs, cores


def _pack_core(lay, core, ef_t):
    """Build one core's packed stream rows for the window sections."""
    R = lay["rows"]
    sbuf = np.zeros((R, 512), dtype=np.float16)
    sel, s_loc, d_glob = core["sel"], core["s_loc"], core["d"]
    bounds, lo_cnt, hi_cnt = core["bounds"], core["lo"], core["hi"]
    for w in range(NW):
        ent = lay["win"][w]
        tl, th, tw, ne, nep = ent["tl"], ent["th"], ent["tw"], ent["ne"], ent["nep"]
        a, b = bounds[w], bounds[w + 1]
        nlo = int(lo_cnt[w]); nhi = int(hi_cnt[w])
        ilo = np.full(tl * 128, PAD_LO, np.int16)
        ilo[:nlo] = _hrow(d_glob[a:a + nlo]).astype(np.int16)
        ihi = np.full(th * 128, PAD_HI, np.int16)
        ihi[:nhi] = (_hrow(d_glob[a + nlo:b]) - NLO).astype(np.int16)
        isrc = np.full(ne, PAD_SRC, np.int16)
        isrc[:nlo] = s_loc[a:a + nlo].astype(np.int16)
        isrc[tl * 128:tl * 128 + nhi] = s_loc[a + nlo:b].astype(np.int16)
        wl = np.full(ne, -512.0, np.float16)
        wl[:nlo] = (s_loc[a:a + nlo] - 128 * w).astype(np.float16)
        wl[tl * 128:tl * 128 + nhi] = (s_loc[a + nlo:b] - 128 * w).astype(np.float16)
        efx = np.zeros((11, nep), np.float16)
        efx[10, :] = 1.0
        efx[:10, :nlo] = ef_t[:, sel[a:a + nlo]]
        efx[:10, tl * 128:tl * 128 + nhi] = ef_t[:, sel[a + nlo:b]]

        wlt = np.zeros((128, ent["twp"]), np.float16)
        wlt[:, :tw] = wl.reshape(tw, 128).T
        ixblk = np.concatenate([
            _wrap_idx(ilo).view(np.float16),
            _wrap_idx(ihi).view(np.float16),
            _wrap_idx(isrc).view(np.float16),
            wlt,
        ], axis=1)                                   # [128, ic]
        assert ixblk.shape[1] == ent["ic"]
        sbuf[ent["ix"]:ent["ix"] + ent["ic"] * 128 // 512] = \
            ixblk.reshape(-1, 512)
        sbuf[ent["eftx"]:ent["eftx"] + 11 * nep // 512] = efx.reshape(-1, 512)
    return sbuf


def kernel(h, edge_index, edge_feat, gate_w, gate_b, cand_w, cand_b,
           bn_gamma, bn_beta):
    global _NC_CACHE, _LAYOUT_CACHE, _LAST_IN_MAPS
    from concourse.bass_utils import run_bass_kernel_spmd

    h = np.asarray(h, dtype=np.float32)
    ei = np.asarray(edge_index)
    src = ei[0].astype(np.int64)
    dst = ei[1].astype(np.int64)
    ef = np.asarray(edge_feat, dtype=np.float32)
    gw = np.asarray(gate_w, dtype=np.float32)
    gb = np.asarray(gate_b, dtype=np.float32)
    cw = np.asarray(cand_w, dtype=np.float32)
    cb = np.asarray(cand_b, dtype=np.float32)
    ef_t = ef.T.astype(np.float16)                       # [10, E]

    TLs, THs, cores = _prep(src, dst, ef_t)
    lay = _layout(TLs, THs)

    wmix = np.concatenate([
        np.concatenate([-gw[0:D], cw[0:D]], axis=1),          # src proj
        np.concatenate([-gw[D:2 * D], cw[D:2 * D]], axis=1),  # dst proj
    ], axis=1).astype(np.float16)                             # [128, 512]
    wefx = np.zeros((11, 256), np.float16)
    wefx[:10] = np.concatenate([-gw[2 * D:], cw[2 * D:]], axis=1)
    wefx[10] = np.concatenate([-gb, cb])                      # bias row
    bnrow = np.concatenate([np.asarray(bn_gamma, np.float32).ravel(),
                            np.asarray(bn_beta, np.float32).ravel()])
    bnrow = bnrow.astype(np.float32).view(np.float16)         # [512]

    in_maps = []
    for k in range(NCORES):
        sbuf = _pack_core(lay, cores[k], ef_t)
        sbuf[lay["wmix"]:lay["wmix"] + 128] = wmix
        sbuf[lay["wefx"]:lay["wefx"] + 11, 0:256] = wefx
        sbuf[lay["bn"]] = bnrow
        hs = np.zeros((HROWS, D), np.float16)
        hs[:NB] = h[k * NB:(k + 1) * NB].astype(np.float16)
        sbuf[lay["h"]:lay["h"] + HROWS * D // 512] = hs.reshape(-1, 512)
        in_maps.append({"stream": sbuf})

    _setup_act_tables()
    if _NC_CACHE is None or _LAYOUT_CACHE != (tuple(TLs), tuple(THs)):
        _NC_CACHE = _build_nc(lay)
        _LAYOUT_CACHE = (tuple(TLs), tuple(THs))
    _LAST_IN_MAPS = in_maps
    res = run_bass_kernel_spmd(_NC_CACHE, in_maps, core_ids=list(range(NCORES)))
    global _LAST_RES
    _LAST_RES = res
    out = np.concatenate([res.results[k]["out"][:NB] for k in range(NCORES)],
                         axis=0)
    return out.astype(np.float32)


if __name__ == "__main__":
    import jax
    import reference
    cpu = jax.devices("cpu")[0]
    with jax.default_device(cpu):
        ins = reference.setup_inputs()
        ins = {k: np.asarray(v) for k, v in ins.items()}
        exp = np.asarray(reference.reference(**{k: jax.device_put(v, cpu) for k, v in ins.items()}))
    got = kernel(**ins)
    err = np.abs(got - exp).max() / np.abs(exp).max()
    print("rel err:", err)
